# revision 47
# baseline (speedup 1.0000x reference)
"""DescendantMax kernel for Trainium2 (8 NeuronCores, pure data parallel).

Tree structure (hardcoded from the problem spec): balanced 8-ary tree,
DEPTH=6 parent->child levels, BFS node numbering.  Level k starts at
s_k = (8^k - 1) / 7 and has 8^k nodes.  Children of the j-th node of
level k are the 8 consecutive nodes s_{k+1} + 8j ... 8j+8.  So each
level's "gather" is a contiguous reshape, and the whole computation per
batch row is a chain of 8:1 contiguous-group max reductions, each
followed by an elementwise max with the parent level's own input values.

Sharding: x is (64, 299593) f32; batch is sharded across 8 cores
(8 rows per core).

Measured facts that drive the structure:
  - SBUF's DMA-port path caps at ~235 GB/s for loads (~315 GB/s
    load+store combined), regardless of queue count.
  - DRAM->DRAM DMA sustains ~610 GB/s and never touches SBUF, so the
    leaf pass-through copy (16.8 of 19.2 MB total) is done D2D while
    SBUF only carries the 8.4 MB leaf read for the reduce + ~2 MB of
    upper levels.
  - Tile round-robins ALL HW-DGE DMA instructions (both trigger
    engines) over 8 semaphore lanes in emission order and allows ONE
    outstanding DMA per lane: instruction N's trigger waits until
    instruction N-8 completes.  So the kernel uses few, large, similar-
    duration DMAs, ordered so the 8-wide in-flight window always mixes
    the D2D copy with the SBUF loads, and all tiny transfers are
    emitted last.

Per-core layout ("row-chunk, c-major"): every level-L tensor (all 8
rows) lives in SBUF as [128, n] where partition q = 8*c + r holds the
c-th contiguous 16th of row r's level-L segment.  Groups of 8 children
stay contiguous inside a partition's free dim at every level down to
level 2, so the level sweep is plain free-dim 8:1 reduces + elementwise
maxes with NO inter-level data movement.  c-major also gives every DRAM
AP an outermost dim of 16, which is what stripes a DMA's packets across
all 16 DMA engines (outer dim 8 would only engage 8 of them).

Levels 1 and 0 finish in an [8, *] row-per-partition layout after a
Tile-tracked DRAM bounce (strided-partition SBUF APs confuse Tile's
dependency tracking).
"""

import numpy as np

BRANCH = 8
DEPTH = 6
BATCH = 64
N_CORES = 8
ROWS = BATCH // N_CORES  # rows per core
# starts[k] = (8^k - 1) // 7 ; starts[DEPTH+1] == total node count
STARTS = [(BRANCH**k - 1) // (BRANCH - 1) for k in range(DEPTH + 2)]
N_NODES = STARTS[DEPTH + 1]  # 299593
CH = 16  # contiguous chunks per row -> partition q = 8*c + r

_cache: dict = {}


def _build_nc():
    import concourse.bacc as bacc
    import concourse.mybir as mybir
    from concourse.tile import TileContext

    f32 = mybir.dt.float32
    AX = mybir.AxisListType.X

    # Bacc (not raw Bass): its compile() pipeline runs
    # generate_event_semaphores, which splits multi-wait sync_info into
    # EventSemaphore insts — TRN2 allows at most 1 wait per instruction.
    nc = bacc.Bacc(None, target_bir_lowering=False)
    x = nc.dram_tensor("x", [ROWS, N_NODES], f32, kind="ExternalInput")
    out = nc.dram_tensor("out", [ROWS, N_NODES], f32, kind="ExternalOutput")

    def cmajor(t, lvl):
        """DRAM AP for level lvl of all rows, enumerated (c, r, f) to
        pair 1:1 with a [128, n] c-major row-chunk SBUF tile."""
        a, b = STARTS[lvl], STARTS[lvl + 1]
        return t[:, a:b].rearrange("r (c f) -> c r f", c=CH)

    LEAF_N = BRANCH**DEPTH // CH  # 16384 leaf elems per partition
    NCHUNK = 8
    CW = LEAF_N // NCHUNK  # 2048 leaf columns per pipeline chunk
    N5 = BRANCH**5 // CH  # 2048

    with TileContext(nc) as tc:
        with (
            # one independent slot per leaf chunk: load triggers never
            # wait on slot-reuse semaphores
            tc.tile_pool(name="big", bufs=NCHUNK) as big,
            tc.tile_pool(name="tail", bufs=1) as tailp,
            tc.tile_pool(name="dram", bufs=1, space="DRAM") as dpool,
        ):
            xls = {5: tailp.tile([128, N5], f32, tag="x5t", name="x5t")}
            m5 = tailp.tile([128, N5], f32)
            m4 = tailp.tile([128, BRANCH**4 // CH], f32)
            leaf_in = cmajor(x, DEPTH)

            def x5load(h):  # one column half of the level-5 x values
                sl5 = slice(h * N5 // 2, (h + 1) * N5 // 2)
                nc.sync.dma_start(
                    out=xls[5][:, sl5], in_=cmajor(x, 5)[:, :, sl5]
                )

            def d2d(j, n):  # leaf pass-through chunk, DRAM->DRAM
                sl = slice(j * LEAF_N // n, (j + 1) * LEAF_N // n)
                nc.scalar.dma_start(
                    out=cmajor(out, DEPTH)[:, :, sl],
                    in_=cmajor(x, DEPTH)[:, :, sl],
                )

            o5s = {}

            def qcompute(q):
                """Level-5 max + next-level reduce for column quarter q
                (DVE only); its output store rides the scalar queue,
                which is idle once the D2D chunks drain."""
                sl5 = slice(q * N5 // 4, (q + 1) * N5 // 4)
                o5 = tailp.tile([128, N5 // 4], f32, tag=f"o5t{q}")
                o5s[q] = o5
                nc.vector.tensor_max(
                    out=o5[:, :], in0=m5[:, sl5], in1=xls[5][:, sl5]
                )
                store_chain.append(
                    nc.scalar.dma_start(
                        out=cmajor(out, 5)[:, :, sl5], in_=o5[:, :]
                    )
                )
                m4sl = slice(q * N5 // 32, (q + 1) * N5 // 32)
                nc.vector.reduce_max(
                    out=m4[:, m4sl],
                    in_=o5[:, :].rearrange("q (g e) -> q g e", e=8),
                    axis=AX,
                )

            # ---- emission order = 8-lane round-robin order ----
            # Leaf chunks 0..NSBST-1 are copied DRAM->DRAM (one 1.05 MB
            # chunk riding before each leaf load, so 1-2 D2D chunks are
            # always in flight alongside the SBUF loads).  The LAST
            # NSBST chunks are instead stored to the output straight
            # from their already-loaded SBUF tiles: their loads finish
            # when the SBUF path is going idle anyway, and skipping the
            # D2D double-read saves 1.05 MB of HBM traffic per chunk.
            # The x5 halves land just before their halfcompute
            # consumers.  The scalar engine's stream is the D2D chunks
            # followed by the (chained) output stores, so no DVE-
            # dependent store can head-of-line block a D2D trigger.
            store_chain = []  # scalar-queue stores, forced into this order
            for k in range(NCHUNK):
                if k == 2:
                    # all four D2D chunks at lane positions 2-5: their
                    # triggers are lane-free, so the whole copy drains
                    # during the load phase instead of after it
                    for j in range(4):
                        d2d(j, 4)
                t6 = big.tile([128, CW], f32, tag="t6")
                nc.sync.dma_start(
                    out=t6[:, :], in_=leaf_in[:, :, k * CW : (k + 1) * CW]
                )
                if k == 1 or k == 5:
                    x5load(0 if k == 1 else 1)
                nc.vector.reduce_max(
                    out=m5[:, k * CW // 8 : (k + 1) * CW // 8],
                    in_=t6[:, :].rearrange("q (g e) -> q g e", e=8),
                    axis=AX,
                )
                if k == 3:
                    qcompute(0)
                    qcompute(1)
                if k == 5:
                    qcompute(2)
            qcompute(3)

            # small per-level x loads (tiny; lanes are clear by now)
            for lvl in (4, 3, 2):
                n = BRANCH**lvl // CH
                xl = tailp.tile([128, n], f32, tag=f"x{lvl}t")
                nc.sync.dma_start(out=xl[:, :], in_=cmajor(x, lvl))
                xls[lvl] = xl
            x1 = tailp.tile([ROWS, 8], f32)
            nc.sync.dma_start(out=x1[:, :], in_=x[:, 1:9])
            x0 = tailp.tile([ROWS, 1], f32)
            nc.sync.dma_start(out=x0[:, :], in_=x[:, 0:1])

            # levels 4 -> 3 -> 2 in c-major row-chunk layout
            prev = m4
            for lvl in (4, 3, 2):
                n = BRANCH**lvl // CH
                o = tailp.tile([128, n], f32, tag=f"o{lvl}t")
                nc.vector.tensor_max(
                    out=o[:, :], in0=prev[:, :], in1=xls[lvl][:, :]
                )
                store_chain.append(
                    nc.sync.dma_start(out=cmajor(out, lvl), in_=o[:, :])
                )
                if lvl > 2:
                    m = tailp.tile([128, n // 8], f32, tag=f"m{lvl - 1}t")
                    nc.vector.reduce_max(
                        out=m[:, :],
                        in_=o[:, :].rearrange("q (g e) -> q g e", e=8),
                        axis=AX,
                    )
                    prev = m

            # repack level-2 output into one-row-per-partition [8, 64] via
            # a Tile-tracked DRAM bounce; the tiny tail chain runs on the
            # sync queue, idle once the leaf loads drain
            d2 = dpool.tile([ROWS, 64], f32)
            nc.sync.dma_start(
                out=d2[:, :].rearrange("r (c f) -> c r f", c=CH), in_=o[:, :]
            )
            t2 = tailp.tile([ROWS, 64], f32)
            nc.sync.dma_start(out=t2[:, :], in_=d2[:, :])
            # level 1
            m1 = tailp.tile([ROWS, 8], f32)
            nc.vector.reduce_max(
                out=m1[:, :],
                in_=t2[:, :].rearrange("q (g e) -> q g e", e=8),
                axis=AX,
            )
            o1 = tailp.tile([ROWS, 8], f32)
            nc.vector.tensor_max(out=o1[:, :], in0=m1[:, :], in1=x1[:, :])
            nc.sync.dma_start(out=out[:, 1:9], in_=o1[:, :])
            # level 0
            m0 = tailp.tile([ROWS, 1], f32)
            nc.vector.reduce_max(
                out=m0[:, :],
                in_=o1[:, :].rearrange("q (g e) -> q g e", e=8),
                axis=AX,
            )
            o0 = tailp.tile([ROWS, 1], f32)
            nc.vector.tensor_max(out=o0[:, :], in0=m0[:, :], in1=x0[:, :])
            nc.sync.dma_start(out=out[:, 0:1], in_=o0[:, :])

            # The Tile scheduler orders each engine's stream by its own
            # priority heuristic, which happily parks a DVE-dependent
            # store in front of the leaf stores and serializes the whole
            # store queue behind the compute chain.  Pin the scalar
            # stream to emission order with explicit scheduling edges.
            import bass_rust as _br

            for a, b in zip(store_chain[1:], store_chain[:-1]):
                _br.add_dep_helper(
                    a.ins, b.ins, False, "keep store-queue order"
                )
    nc.compile()
    return nc


def _get_nc():
    if "nc" not in _cache:
        _cache["nc"] = _build_nc()
    return _cache["nc"]


def kernel(x, level_parents=None, level_children=None, **_ignored):
    from concourse.bass_utils import run_bass_kernel_spmd

    x = np.ascontiguousarray(np.asarray(x), dtype=np.float32)
    assert x.shape == (BATCH, N_NODES), x.shape

    nc = _get_nc()
    core_ids = list(range(N_CORES))
    in_maps = [
        {"x": x[i * ROWS : (i + 1) * ROWS]} for i in range(N_CORES)
    ]
    res = run_bass_kernel_spmd(nc, in_maps, core_ids)
    return np.concatenate([res.results[i]["out"] for i in range(N_CORES)], axis=0)


# revision 48
# speedup vs baseline: 1.0740x; 1.0740x over previous
"""DescendantMax kernel for Trainium2 (8 NeuronCores, pure data parallel).

Tree structure (hardcoded from the problem spec): balanced 8-ary tree,
DEPTH=6 parent->child levels, BFS node numbering.  Level k starts at
s_k = (8^k - 1) / 7 and has 8^k nodes.  Children of the j-th node of
level k are the 8 consecutive nodes s_{k+1} + 8j ... 8j+8.  So each
level's "gather" is a contiguous reshape, and the whole computation per
batch row is a chain of 8:1 contiguous-group max reductions, each
followed by an elementwise max with the parent level's own input values.

Sharding: x is (64, 299593) f32; batch is sharded across 8 cores
(8 rows per core).

Measured facts that drive the structure:
  - SBUF's DMA-port path caps at ~235 GB/s for loads (~315 GB/s
    load+store combined), regardless of queue count.
  - DRAM->DRAM DMA sustains ~610 GB/s and never touches SBUF, so the
    leaf pass-through copy (16.8 of 19.2 MB total) is done D2D while
    SBUF only carries the 8.4 MB leaf read for the reduce + ~2 MB of
    upper levels.
  - Tile round-robins ALL HW-DGE DMA instructions (both trigger
    engines) over 8 semaphore lanes in emission order and allows ONE
    outstanding DMA per lane: instruction N's trigger waits until
    instruction N-8 completes.  So the kernel uses few, large, similar-
    duration DMAs, ordered so the 8-wide in-flight window always mixes
    the D2D copy with the SBUF loads, and all tiny transfers are
    emitted last.

Per-core layout ("row-chunk, c-major"): every level-L tensor (all 8
rows) lives in SBUF as [128, n] where partition q = 8*c + r holds the
c-th contiguous 16th of row r's level-L segment.  Groups of 8 children
stay contiguous inside a partition's free dim at every level down to
level 2, so the level sweep is plain free-dim 8:1 reduces + elementwise
maxes with NO inter-level data movement.  c-major also gives every DRAM
AP an outermost dim of 16, which is what stripes a DMA's packets across
all 16 DMA engines (outer dim 8 would only engage 8 of them).

Levels 1 and 0 finish in an [8, *] row-per-partition layout after a
Tile-tracked DRAM bounce (strided-partition SBUF APs confuse Tile's
dependency tracking).
"""

import numpy as np

BRANCH = 8
DEPTH = 6
BATCH = 64
N_CORES = 8
ROWS = BATCH // N_CORES  # rows per core
# starts[k] = (8^k - 1) // 7 ; starts[DEPTH+1] == total node count
STARTS = [(BRANCH**k - 1) // (BRANCH - 1) for k in range(DEPTH + 2)]
N_NODES = STARTS[DEPTH + 1]  # 299593
CH = 16  # contiguous chunks per row -> partition q = 8*c + r

_cache: dict = {}


def _build_nc():
    import concourse.bacc as bacc
    import concourse.mybir as mybir
    from concourse.tile import TileContext

    f32 = mybir.dt.float32
    AX = mybir.AxisListType.X

    # Bacc (not raw Bass): its compile() pipeline runs
    # generate_event_semaphores, which splits multi-wait sync_info into
    # EventSemaphore insts — TRN2 allows at most 1 wait per instruction.
    nc = bacc.Bacc(None, target_bir_lowering=False)
    x = nc.dram_tensor("x", [ROWS, N_NODES], f32, kind="ExternalInput")
    out = nc.dram_tensor("out", [ROWS, N_NODES], f32, kind="ExternalOutput")

    def cmajor(t, lvl):
        """DRAM AP for level lvl of all rows, enumerated (c, r, f) to
        pair 1:1 with a [128, n] c-major row-chunk SBUF tile."""
        a, b = STARTS[lvl], STARTS[lvl + 1]
        return t[:, a:b].rearrange("r (c f) -> c r f", c=CH)

    LEAF_N = BRANCH**DEPTH // CH  # 16384 leaf elems per partition
    NCHUNK = 8
    CW = LEAF_N // NCHUNK  # 2048 leaf columns per pipeline chunk
    N5 = BRANCH**5 // CH  # 2048

    with TileContext(nc) as tc:
        with (
            # one independent slot per leaf chunk: load triggers never
            # wait on slot-reuse semaphores
            tc.tile_pool(name="big", bufs=NCHUNK) as big,
            tc.tile_pool(name="tail", bufs=1) as tailp,
            tc.tile_pool(name="dram", bufs=1, space="DRAM") as dpool,
        ):
            xls = {5: tailp.tile([128, N5], f32, tag="x5t", name="x5t")}
            m5 = tailp.tile([128, N5], f32)
            m4 = tailp.tile([128, BRANCH**4 // CH], f32)
            leaf_in = cmajor(x, DEPTH)

            def x5load(h):  # one column half of the level-5 x values
                sl5 = slice(h * N5 // 2, (h + 1) * N5 // 2)
                nc.sync.dma_start(
                    out=xls[5][:, sl5], in_=cmajor(x, 5)[:, :, sl5]
                )

            def d2d(j, n):  # leaf pass-through chunk, DRAM->DRAM
                sl = slice(j * LEAF_N // n, (j + 1) * LEAF_N // n)
                nc.scalar.dma_start(
                    out=cmajor(out, DEPTH)[:, :, sl],
                    in_=cmajor(x, DEPTH)[:, :, sl],
                )

            o5s = {}

            def halfcompute(h):
                """Level-5 max + next-level reduce for column half h
                (DVE only; the store is emitted after the load stream)."""
                sl5 = slice(h * N5 // 2, (h + 1) * N5 // 2)
                o5 = tailp.tile([128, N5 // 2], f32, tag=f"o5t{h}")
                o5s[h] = o5
                nc.vector.tensor_max(
                    out=o5[:, :], in0=m5[:, sl5], in1=xls[5][:, sl5]
                )
                m4sl = slice(h * N5 // 16, (h + 1) * N5 // 16)
                nc.vector.reduce_max(
                    out=m4[:, m4sl],
                    in_=o5[:, :].rearrange("q (g e) -> q g e", e=8),
                    axis=AX,
                )

            # ---- emission order = 8-lane round-robin order ----
            # Leaf chunks 0..NSBST-1 are copied DRAM->DRAM (one 1.05 MB
            # chunk riding before each leaf load, so 1-2 D2D chunks are
            # always in flight alongside the SBUF loads).  The LAST
            # NSBST chunks are instead stored to the output straight
            # from their already-loaded SBUF tiles: their loads finish
            # when the SBUF path is going idle anyway, and skipping the
            # D2D double-read saves 1.05 MB of HBM traffic per chunk.
            # The x5 halves land just before their halfcompute
            # consumers.  The scalar engine's stream is the D2D chunks
            # followed by the (chained) output stores, so no DVE-
            # dependent store can head-of-line block a D2D trigger.
            store_chain = []  # scalar-queue stores, forced into this order
            for k in range(NCHUNK):
                if k == 2:
                    # all four D2D chunks at lane positions 2-5: their
                    # triggers are lane-free, so the whole copy drains
                    # during the load phase instead of after it
                    for j in range(4):
                        d2d(j, 4)
                t6 = big.tile([128, CW], f32, tag="t6")
                nc.sync.dma_start(
                    out=t6[:, :], in_=leaf_in[:, :, k * CW : (k + 1) * CW]
                )
                if k == 1 or k == 5:
                    x5load(0 if k == 1 else 1)
                nc.vector.reduce_max(
                    out=m5[:, k * CW // 8 : (k + 1) * CW // 8],
                    in_=t6[:, :].rearrange("q (g e) -> q g e", e=8),
                    axis=AX,
                )
                if k == NCHUNK // 2 - 1:
                    halfcompute(0)  # overlaps leaf chunks 4-7
            halfcompute(1)

            # small per-level x loads (tiny; lanes are clear by now)
            for lvl in (4, 3, 2):
                n = BRANCH**lvl // CH
                xl = tailp.tile([128, n], f32, tag=f"x{lvl}t")
                nc.sync.dma_start(out=xl[:, :], in_=cmajor(x, lvl))
                xls[lvl] = xl
            x1 = tailp.tile([ROWS, 8], f32)
            nc.sync.dma_start(out=x1[:, :], in_=x[:, 1:9])
            x0 = tailp.tile([ROWS, 1], f32)
            nc.sync.dma_start(out=x0[:, :], in_=x[:, 0:1])

            # level-5 output stores, on scalar behind the leaf stores
            for h in (0, 1):
                sl5 = slice(h * N5 // 2, (h + 1) * N5 // 2)
                store_chain.append(
                    nc.sync.dma_start(
                        out=cmajor(out, 5)[:, :, sl5], in_=o5s[h][:, :]
                    )
                )

            # levels 4 -> 3 -> 2 in c-major row-chunk layout
            prev = m4
            for lvl in (4, 3, 2):
                n = BRANCH**lvl // CH
                o = tailp.tile([128, n], f32, tag=f"o{lvl}t")
                nc.vector.tensor_max(
                    out=o[:, :], in0=prev[:, :], in1=xls[lvl][:, :]
                )
                store_chain.append(
                    nc.sync.dma_start(out=cmajor(out, lvl), in_=o[:, :])
                )
                if lvl > 2:
                    m = tailp.tile([128, n // 8], f32, tag=f"m{lvl - 1}t")
                    nc.vector.reduce_max(
                        out=m[:, :],
                        in_=o[:, :].rearrange("q (g e) -> q g e", e=8),
                        axis=AX,
                    )
                    prev = m

            # repack level-2 output into one-row-per-partition [8, 64] via
            # a Tile-tracked DRAM bounce; the tiny tail chain runs on the
            # sync queue, idle once the leaf loads drain
            d2 = dpool.tile([ROWS, 64], f32)
            nc.sync.dma_start(
                out=d2[:, :].rearrange("r (c f) -> c r f", c=CH), in_=o[:, :]
            )
            t2 = tailp.tile([ROWS, 64], f32)
            nc.sync.dma_start(out=t2[:, :], in_=d2[:, :])
            # level 1
            m1 = tailp.tile([ROWS, 8], f32)
            nc.vector.reduce_max(
                out=m1[:, :],
                in_=t2[:, :].rearrange("q (g e) -> q g e", e=8),
                axis=AX,
            )
            o1 = tailp.tile([ROWS, 8], f32)
            nc.vector.tensor_max(out=o1[:, :], in0=m1[:, :], in1=x1[:, :])
            nc.sync.dma_start(out=out[:, 1:9], in_=o1[:, :])
            # level 0
            m0 = tailp.tile([ROWS, 1], f32)
            nc.vector.reduce_max(
                out=m0[:, :],
                in_=o1[:, :].rearrange("q (g e) -> q g e", e=8),
                axis=AX,
            )
            o0 = tailp.tile([ROWS, 1], f32)
            nc.vector.tensor_max(out=o0[:, :], in0=m0[:, :], in1=x0[:, :])
            nc.sync.dma_start(out=out[:, 0:1], in_=o0[:, :])

            # The Tile scheduler orders each engine's stream by its own
            # priority heuristic, which happily parks a DVE-dependent
            # store in front of the leaf stores and serializes the whole
            # store queue behind the compute chain.  Pin the scalar
            # stream to emission order with explicit scheduling edges.
            import bass_rust as _br

            for a, b in zip(store_chain[1:], store_chain[:-1]):
                _br.add_dep_helper(
                    a.ins, b.ins, False, "keep store-queue order"
                )
    nc.compile()
    return nc


def _get_nc():
    if "nc" not in _cache:
        _cache["nc"] = _build_nc()
    return _cache["nc"]


def kernel(x, level_parents=None, level_children=None, **_ignored):
    from concourse.bass_utils import run_bass_kernel_spmd

    x = np.ascontiguousarray(np.asarray(x), dtype=np.float32)
    assert x.shape == (BATCH, N_NODES), x.shape

    nc = _get_nc()
    core_ids = list(range(N_CORES))
    in_maps = [
        {"x": x[i * ROWS : (i + 1) * ROWS]} for i in range(N_CORES)
    ]
    res = run_bass_kernel_spmd(nc, in_maps, core_ids)
    return np.concatenate([res.results[i]["out"] for i in range(N_CORES)], axis=0)
